# revision 52
# baseline (speedup 1.0000x reference)
"""Trainium2 Bass kernel for BaseMultiheadAttention (bf16, ~200us/core).

dims: B=1, V=4, S=2048, E=512, H=8, D=64 (head_dim), causal, interleaved RoPE.

Sharding (8 cores): core c -> bv index g = c//2, head-group hg = c%2
(4 heads each).  Each core computes its bv-slice's QKV projection restricted
to its 4 heads, RoPE, causal attention, and a partial output projection
(its heads' wO rows).  Host sums the two bf16 partials per bv index.

Design (evolved from the 283us fp32r baseline via NTFF profiling):
  - all matmul operands bf16: halves LDWEIGHTS time, DVE ops and DMA bytes
    (fp32r matmul column rate is already 1 col/cycle at >=256 moving cols,
    so the win is in weight loads / SBUF pressure, not column throughput)
  - the dominant limiter is the PE p-state governor (0.65/1.2/2.4 GHz; full
    clock only after ~3us of gapless execution), so everything is organized
    around keeping the tensor engine's queue non-empty:
      * ONE global software-pipelined step stream over all eight
        (head-stack, q-block) attention blocks: scores(step+2) issue ahead
        of PV(step), so PV never waits on the scalar-engine exp
      * stack-1 q/k projections+rope, softmax-normalize tails and the
        output projection are queued as single-thunk "fillers" drained one
        per step inside the attention stream (PE gap filler)
      * per-block PSUM PV accumulators are freed immediately via one DVE
        cast to SBUF (praw); 1/denom = Exp(-Ln(denom)) runs on the scalar
        engine ([1,1024] row, the ones-column of the 65-row PV lhsT
        accumulates denominators for free), and the broadcast ones-matmul +
        normalize multiplies are deferred fillers
  - causal diag masking by -1e30 psum prefill (identity x trib matmul)
    ahead of the scores accumulation - off the exp->PV dependency chain
  - PSUM map (8 banks): "big" [128,1024] x3 rotating (scores / proj /
    outproj / denom broadcast) + "po" [128,1024] x1 (PV accumulator)
  - input DMAs split by first-need across the sync/gpsimd/scalar queues
"""

import numpy as np

import concourse.bass as bass
import concourse.mybir as mybir
from concourse.tile import TileContext

# ---- problem dims (hardcoded per the task contract) ----
B, V, S, E, H = 1, 4, 2048, 512, 8
D = E // H            # 64
HG = 4                # heads per core
NCORE = 8
F32 = mybir.dt.float32
BF = mybir.dt.bfloat16


def _bf16(a):
    import ml_dtypes
    return np.asarray(a, np.float32).astype(ml_dtypes.bfloat16)


def _host_tables():
    pos = np.arange(S, dtype=np.float64)
    inv_freq = 1.0 / (10000.0 ** (np.arange(0, D, 2, dtype=np.float64) / D))
    freqs = pos[:, None] * inv_freq[None, :]          # (S, D/2)
    freqs = np.repeat(freqs, 2, axis=-1)              # (S, D) interleaved
    cosT = np.cos(freqs).T.astype(np.float32)         # (D, S)
    sinT = np.sin(freqs).T.astype(np.float32)
    cs = np.concatenate([cosT, cosT], axis=0)         # (128, S) two-head stack
    sn = np.concatenate([sinT, sinT], axis=0)
    sgn = np.tile(np.array([-1.0, 1.0], np.float32), D // 2)[:, None]
    snS = sn * np.concatenate([sgn, sgn], axis=0)
    # -inf prefill for the 128-wide diag block (rows=k, cols=q): mask q < k,
    # padded to 512 with zeros (the strictly-lower remainder of the stream)
    trib = np.zeros((128, 512), np.float32)
    for p in range(128):
        trib[p, :p] = -1.0e30
    iden = np.eye(128, dtype=np.float32)
    return cs, snS, trib, iden


def _host_weights(wqkv_w, wqkv_b, wo_w, hg):
    """Per-head-group weight slices in the kernel's layouts (bf16)."""
    heads = [hg * HG + h for h in range(HG)]
    cs, snS, trib, iden = _host_tables()
    # feature index inside each qkv block: d*H + h
    def rows(block, h):
        d = np.arange(D)
        return block * E + d * H + h
    def to_T(Wh):   # (HG, D, E) -> (E, HG*D) with col = h*64+d
        return np.transpose(Wh, (2, 0, 1)).reshape(E, HG * D).astype(np.float32)
    Wq = np.stack([wqkv_w[rows(0, h)] for h in heads])   # (HG, D, E)
    Wk = np.stack([wqkv_w[rows(1, h)] for h in heads])
    Wv = np.stack([wqkv_w[rows(2, h)] for h in heads])
    wqkvT = np.concatenate([to_T(Wq), to_T(Wk), to_T(Wv)], axis=1)  # (E, 768)
    # wo rows for this head group: out feature = h_global*64 + d
    woT = np.stack([wo_w[:, (hg * HG + h) * D:(hg * HG + h + 1) * D].T
                    for h in range(HG)])                 # (HG, D, E)
    woT = woT.reshape(HG * D, E).astype(np.float32)      # (256, 512)
    return dict(wqkvT=_bf16(wqkvT), woT=_bf16(woT),
                cs=_bf16(cs), snS=_bf16(snS), trib=_bf16(trib),
                iden=_bf16(iden))


_MAX_WAITS = {"Matmult": 1}          # per-opcode cap; default below
_DEF_MAX_WAITS = 1


def _split_excess_waits(nc):
    """This walrus build encodes at most ~1 sync-wait per instruction.
    Post-process the serialized BIR: hoist excess on_wait entries onto
    same-engine NoOp carriers emitted immediately before the instruction."""
    import orjson

    orig = nc.to_json_bytes

    def patched(_self=None):
        d = orjson.loads(orig())
        for fn in d.get("functions", []):
            for bb in fn.get("basicblocks", fn.get("blocks", [])):
                insts = bb.get("instructions")
                if insts is None:
                    continue
                out, nctr = [], 0
                for inst in insts:
                    si = inst.get("sync_info")
                    waits = (si or {}).get("on_wait") or []
                    cap = _MAX_WAITS.get(inst.get("opcode"), _DEF_MAX_WAITS)
                    if len(waits) > cap:
                        keep = waits[:cap]
                        extra = waits[cap:]
                        for w in extra:
                            nctr += 1
                            out.append({
                                "debug": inst.get("debug", 0),
                                "engine": inst["engine"],
                                "ins": [], "outs": [],
                                "name": f"{inst['name']}_w{nctr}",
                                "opcode": "NoOp",
                                "sync_info": {"on_wait": [w],
                                              "on_update": []},
                            })
                        si["on_wait"] = keep
                    out.append(inst)
                bb["instructions"] = out
        return orjson.dumps(d)

    nc.to_json_bytes = patched
    return nc


def build_nc(reps=1, debug=False):
    nc = bass.Bass()
    xT = nc.declare_dram_parameter("xT", [E, S], BF, isOutput=False)
    wqkvT = nc.declare_dram_parameter("wqkvT", [E, 3 * HG * D], BF,
                                      isOutput=False)
    woT = nc.declare_dram_parameter("woT", [HG * D, E], BF, isOutput=False)
    cs = nc.declare_dram_parameter("cs", [128, S], BF, isOutput=False)
    snS = nc.declare_dram_parameter("snS", [128, S], BF, isOutput=False)
    trib = nc.declare_dram_parameter("trib", [128, 512], BF, isOutput=False)
    iden = nc.declare_dram_parameter("iden", [128, 128], BF, isOutput=False)
    outT = nc.declare_dram_parameter("outT", [E, S], BF, isOutput=True)
    if debug:
        dbg_qrot = nc.declare_dram_parameter("dbg_qrot", [128, S], BF,
                                             isOutput=True)
        dbg_krot = nc.declare_dram_parameter("dbg_krot", [128, S], BF,
                                             isOutput=True)
        dbg_vt = nc.declare_dram_parameter("dbg_vt", [128, HG * 65], BF,
                                           isOutput=True)
        dbg_on = nc.declare_dram_parameter("dbg_on", [128, 512], BF,
                                           isOutput=True)

    SWAP = [1, 0, 3, 2, 5, 4, 7, 6, 9, 8, 11, 10, 13, 12, 15, 14,
            17, 16, 19, 18, 21, 20, 23, 22, 25, 24, 27, 26, 29, 28, 31, 30]
    scale = 1.0 / np.sqrt(D)

    with TileContext(nc) as tc:
      for _rep in range(reps):
        with (
            tc.tile_pool(name="const", bufs=1) as cpool,
            tc.tile_pool(name="qk", bufs=1) as qkpool,
            tc.tile_pool(name="v", bufs=1) as vpool,
            tc.tile_pool(name="pt", bufs=3) as ptpool,
            tc.tile_pool(name="rope", bufs=3) as rpool,
            tc.tile_pool(name="on", bufs=1) as onpool,
            tc.tile_pool(name="sums", bufs=2) as spool,
            tc.tile_pool(name="oc", bufs=3) as ocpool,
        ):
            # ---- input DMAs, first-needed-first, spread over the three
            # DMA-capable queues (sync / gpsimd / scalar) ----
            xt = [cpool.tile([128, S], BF, tag=f"xt{e4}", name=f"xt{e4}")
                  for e4 in range(4)]
            w_t = [cpool.tile([128, 3 * HG * D], BF, tag=f"w{e4}",
                              name=f"w{e4}") for e4 in range(4)]
            cs_t = cpool.tile([128, S], BF, tag="cs", name="cs_t")
            sn_t = cpool.tile([128, S], BF, tag="sn", name="sn_t")
            trib_t = cpool.tile([128, 512], BF, tag="trib", name="trib_t")
            iden_t = cpool.tile([128, 128], BF, tag="iden", name="iden_t")
            wo_t = [cpool.tile([128, E], BF, tag=f"wo{f2}", name=f"wo{f2}")
                    for f2 in range(2)]
            # k-weight chunks (first projection) then rope tables
            for e4 in range(4):
                nc.scalar.dma_start(
                    w_t[e4][:, HG * D:2 * HG * D],
                    wqkvT[e4 * 128:(e4 + 1) * 128, HG * D:2 * HG * D])
            # x chunks, block-interleaved so early s-blocks land first
            for blk in range(4):
                for e4 in range(4):
                    eng = nc.sync if (e4 % 2 == 0) else nc.gpsimd
                    eng.dma_start(
                        xt[e4][:, blk * 512:(blk + 1) * 512],
                        xT[e4 * 128:(e4 + 1) * 128,
                           blk * 512:(blk + 1) * 512])
            nc.scalar.dma_start(cs_t[:, :], cs[:, :])
            nc.scalar.dma_start(sn_t[:, :], snS[:, :])
            for e4 in range(4):   # q-weight chunks
                nc.scalar.dma_start(
                    w_t[e4][:, 0:HG * D],
                    wqkvT[e4 * 128:(e4 + 1) * 128, 0:HG * D])
            for e4 in range(4):   # v-weight chunks
                nc.scalar.dma_start(
                    w_t[e4][:, 2 * HG * D:3 * HG * D],
                    wqkvT[e4 * 128:(e4 + 1) * 128, 2 * HG * D:3 * HG * D])
            nc.scalar.dma_start(trib_t[:, :], trib[:, :])
            nc.scalar.dma_start(iden_t[:, :], iden[:, :])
            for f2 in range(2):
                nc.scalar.dma_start(wo_t[f2][:, :],
                                    woT[f2 * 128:(f2 + 1) * 128, :])
            ones_b = cpool.tile([1, D], BF, tag="ones", name="ones_b")
            nc.vector.memset(ones_b[:, :], 1.0)

            qrot, krot = {}, {}
            onorm = {}
            v_t = []
            with tc.tile_pool(name="psA", bufs=1, space="PSUM") as psA:
                # PSUM bank budget (8 banks of [128,512]f32):
                #   duo  [128,1024] x2 bufs = 4 banks (scores)
                #   po   [128,1024] x1 buf  = 2 banks (PV accum, both heads)
                #   sm   [128,512]  x2 bufs = 2 banks (qkv proj, rope, outproj)

                filler = []     # queued single-matmul thunks (PE gap filler)

                def drain_filler(n=1):
                    for _ in range(n):
                        if filler:
                            filler.pop(0)()

                def project_rope(tgt, base, dst, st, queued=False):
                    """Q/K projection + rotary for stack st (2 heads)."""
                    rt = qkpool.tile([128, S], BF, tag=f"{tgt}rot{st}",
                                     name=f"{tgt}rot{st}")
                    dst[st] = rt
                    fcol = base + st * 2 * D

                    def mk(blk):
                        def f():
                            s0 = blk * 512
                            pq = psA.tile([128, 1024], F32, tag="big",
                                          bufs=3, name="pq")
                            for e4 in range(4):
                                nc.tensor.matmul(
                                    pq[:, 0:512],
                                    w_t[e4][:, fcol:fcol + 128],
                                    xt[e4][:, s0:s0 + 512],
                                    start=(e4 == 0), stop=(e4 == 3))
                            sh_t = rpool.tile([128, 512], F32, tag="ropesh",
                                              name="ropesh")
                            t1 = rpool.tile([128, 512], BF, tag="ropet1",
                                            name="ropet1")
                            t2 = rpool.tile([128, 512], BF, tag="ropet2",
                                            name="ropet2")
                            nc.vector.stream_shuffle(sh_t[:, :], pq[:, 0:512],
                                                     SWAP)
                            nc.vector.tensor_mul(
                                t1[:, :], pq[:, 0:512], cs_t[:, s0:s0 + 512])
                            nc.vector.tensor_mul(
                                t2[:, :], sh_t[:, :], sn_t[:, s0:s0 + 512])
                            nc.vector.tensor_add(
                                rt[:, s0:s0 + 512], t1[:, :], t2[:, :])
                        return f

                    for blk in range(4):
                        f = mk(blk)
                        if queued:
                            filler.append(f)
                        else:
                            f()

                def project_v(i0, i1, queued=False):
                    def mk(i):
                        def f():
                            vt = vpool.tile([128, HG * 65], BF, tag=f"v{i}",
                                            name=f"v{i}")
                            assert len(v_t) == i
                            v_t.append(vt)
                            pv = psA.tile([128, 1024], F32, tag="big",
                                          bufs=3, name="pvb")
                            for e4 in range(4):
                                nc.tensor.matmul(
                                    pv[:, 0:HG * D],
                                    xt[e4][:, i * 128:(i + 1) * 128],
                                    w_t[e4][:, 2 * HG * D:3 * HG * D],
                                    start=(e4 == 0), stop=(e4 == 3))
                            vt3 = vt[:, :].rearrange("p (h x) -> p h x",
                                                     h=HG)
                            nc.vector.memset(vt3[:, :, D:D + 1], 1.0)
                            nc.vector.tensor_copy(
                                vt3[:, :, 0:D],
                                pv[:, 0:HG * D].rearrange("p (h d) -> p h d",
                                                          h=HG))
                        return f
                    for i in range(i0, i1):
                        f = mk(i)
                        if queued:
                            filler.append(f)
                        else:
                            f()

                def emit_scores(hp, j, i):
                    """Scores + exp for k-tile i of q-block j.  Diag tiles
                    are causal-masked by prefilling the psum with -1e30
                    above the diagonal (identity matmul on trib)."""
                    r = i - 4 * j
                    offs = 128 * r if r >= 0 else 0
                    sc = psA.tile([128, 1024], F32, tag="big", bufs=3,
                                  name="sc")
                    pt = ptpool.tile([128, 1024], BF, tag="pt", bufs=5,
                                     name="pt")
                    for hh in range(2):
                        nc.tensor.matmul(
                            sc[:, hh * 512 + offs:hh * 512 + 512],
                            krot[hp][hh * D:hh * D + D,
                                     i * 128:(i + 1) * 128],
                            qrot[hp][hh * D:hh * D + D,
                                     j * 512 + offs:(j + 1) * 512],
                            start=True, stop=(r < 0))
                        if r >= 0:
                            nc.tensor.matmul(
                                sc[:, hh * 512 + offs:hh * 512 + offs + 128],
                                iden_t[:, :], trib_t[:, 0:128],
                                start=False, stop=True)
                    src = sc[:, :].rearrange("p (h x) -> p h x",
                                             h=2)[:, :, offs:512]
                    dstv = pt[:, :].rearrange("p (h x) -> p h x",
                                              h=2)[:, :, offs:512]
                    nc.scalar.activation(
                        dstv, src, mybir.ActivationFunctionType.Exp,
                        scale=float(scale))
                    return pt, offs

                def emit_pv(hp, j, i, pt, offs, n_i):
                    if i == 0:
                        po_cur[0] = psA.tile([128, 1024], F32, tag="po",
                                             bufs=1, name="po")
                    po = po_cur[0]
                    for hh in range(2):
                        h = 2 * hp + hh
                        nc.tensor.matmul(
                            po[0:65, hh * 512 + offs:hh * 512 + 512],
                            v_t[i][:, h * 65:h * 65 + 65],
                            pt[:, hh * 512 + offs:hh * 512 + 512],
                            start=(i == 0), stop=(i == n_i - 1))
                    if i == n_i - 1:
                        finish_block(hp, j, po)

                po_cur = [None]

                def queue_outproj(j):
                    """Push outproj(j) as 4 filler thunks (one per e-chunk)."""
                    def mk(j, eb):
                        def f():
                            pp = psA.tile([128, 1024], F32, tag="big",
                                          bufs=3, name="pp")
                            for f2 in range(2):
                                nc.tensor.matmul(
                                    pp[:, 0:512],
                                    wo_t[f2][:, eb * 128:(eb + 1) * 128],
                                    onorm[(j, f2)][:, :],
                                    start=(f2 == 0), stop=(f2 == 1))
                            oc = ocpool.tile([128, 512], BF,
                                             tag="oc", name="oc")
                            nc.vector.tensor_copy(oc[:, :], pp[:, 0:512])
                            nc.sync.dma_start(
                                outT[eb * 128:(eb + 1) * 128,
                                     j * 512:(j + 1) * 512],
                                oc[:, :])
                        return f
                    for eb in range(4):
                        filler.append(mk(j, eb))

                def queue_norm_tail(j, hp, praw, rrow_b):
                    """Deferred: broadcast 1/denom via ones-matmul into a
                    fresh duo-psum tile, multiply praw by it into onorm."""
                    def f():
                        onj = onpool.tile([128, 512], BF, tag=f"on{j}{hp}",
                                          name=f"on{j}{hp}")
                        onorm[(j, hp)] = onj
                        bc = psA.tile([128, 1024], F32, tag="big", bufs=3,
                                      name="bc")
                        for hh in range(2):
                            nc.tensor.matmul(
                                bc[0:D, hh * 512:hh * 512 + 512],
                                ones_b[:, :],
                                rrow_b[:, hh * 512:hh * 512 + 512],
                                start=True, stop=True)
                        for hh in range(2):
                            nc.vector.tensor_mul(
                                onj[hh * D:hh * D + D, :],
                                praw[0:D, hh * 512:hh * 512 + 512],
                                bc[0:D, hh * 512:hh * 512 + 512])
                        if debug and j == 1 and hp == 0:
                            nc.sync.dma_start(dbg_on[:, :], onj[:, :])
                        if hp == 1:
                            queue_outproj(j)
                    filler.append(f)

                def finish_block(hp, j, po):
                    # free po fast: copy raw PV (rows 0:64 per head + denom
                    # row 64) to SBUF, compute 1/denom = Exp(-Ln(denom)) on
                    # the scalar engine; the broadcast + normalize multiply
                    # are deferred into the filler queue
                    praw = spool.tile([65, 1024], BF, tag=f"praw{j}{hp}",
                                      name="praw")
                    nc.vector.tensor_copy(praw[:, :], po[0:65, :])
                    rln = spool.tile([1, 1024], F32, tag="rln", name="rln")
                    rrow_b = spool.tile([1, 1024], BF, tag=f"rrowb{j}{hp}",
                                        name="rrowb")
                    nc.scalar.activation(rln[:, :], praw[64:65, :],
                                         mybir.ActivationFunctionType.Ln)
                    nc.scalar.activation(rrow_b[:, :], rln[:, :],
                                         mybir.ActivationFunctionType.Exp,
                                         scale=-1.0)
                    queue_norm_tail(j, hp, praw, rrow_b)

                # schedule: stack-1 projections, normalize tails and outproj
                # run as fillers inside one GLOBAL software-pipelined step
                # stream (scores lookahead 2, PV behind) spanning all eight
                # (stack, q-block) attention blocks, so the PE never drains
                project_rope("k", HG * D, krot, 0)
                project_rope("q", 0, qrot, 0)
                project_v(0, 4)
                project_v(4, S // 128, queued=True)
                project_rope("k", HG * D, krot, 1, queued=True)
                project_rope("q", 0, qrot, 1, queued=True)
                SCHED = ((0, 1), (0, 2), (0, 3), (1, 1), (0, 0),
                         (1, 2), (1, 0), (1, 3))
                steps = [(hp, j, i) for hp, j in SCHED
                         for i in range(4 * j + 4)]
                pts = {}
                for idx in range(len(steps)):
                    if idx == 0:
                        pts[steps[0]] = emit_scores(*steps[0])
                        pts[steps[1]] = emit_scores(*steps[1])
                    if idx + 2 < len(steps):
                        pts[steps[idx + 2]] = emit_scores(*steps[idx + 2])
                    drain_filler(2)
                    hp, j, i = steps[idx]
                    pt, offs = pts.pop((hp, j, i))
                    emit_pv(hp, j, i, pt, offs, 4 * j + 4)
                while filler:
                    filler.pop(0)()
                if debug:
                    nc.sync.dma_start(dbg_qrot[:, :], qrot[0][:, :])
                    nc.sync.dma_start(dbg_krot[:, :], krot[0][:, :])
                    nc.sync.dma_start(dbg_vt[:, :], v_t[0][:, :])
    return _split_excess_waits(nc)


_NC_CACHE = {}


def _get_nc(reps=1):
    if reps not in _NC_CACHE:
        _NC_CACHE[reps] = build_nc(reps)
    return _NC_CACHE[reps]


_RUNNER_CACHE = {}


def _get_runner(nc, n_cores):
    """Clone of bass2jax.run_bass_via_pjrt's multi-core path with the
    jitted callable cached so repeat calls skip retracing."""
    key = id(nc)
    if key in _RUNNER_CACHE:
        return _RUNNER_CACHE[key]
    import jax
    from jax.sharding import Mesh, PartitionSpec
    from jax.experimental.shard_map import shard_map
    from concourse import bass2jax as b2j

    b2j.install_neuronx_cc_hook()
    partition_name = (nc.partition_id_tensor.name
                      if nc.partition_id_tensor else None)
    in_names, out_names, out_avals, zero_outs = [], [], [], []
    for alloc in nc.m.functions[0].allocations:
        if not isinstance(alloc, mybir.MemoryLocationSet):
            continue
        name = alloc.memorylocations[0].name
        if alloc.kind == "ExternalInput":
            if name != partition_name:
                in_names.append(name)
        elif alloc.kind == "ExternalOutput":
            shape = tuple(alloc.tensor_shape)
            dtype = mybir.dt.np(alloc.dtype)
            out_names.append(name)
            out_avals.append(jax.core.ShapedArray(shape, dtype))
            zero_outs.append(np.zeros(shape, dtype))
    n_params = len(in_names)
    n_outs = len(out_avals)
    in_names_all = list(in_names) + list(out_names)
    if partition_name is not None:
        in_names_all.append(partition_name)
    donate = tuple(range(n_params, n_params + n_outs))

    def _body(*args):
        operands = list(args)
        if partition_name is not None:
            operands.append(b2j.partition_id_tensor())
        outs = b2j._bass_exec_p.bind(
            *operands,
            out_avals=tuple(out_avals),
            in_names=tuple(in_names_all),
            out_names=tuple(out_names),
            lowering_input_output_aliases=(),
            sim_require_finite=True,
            sim_require_nnan=True,
            nc=nc,
        )
        return tuple(outs)

    devices = jax.devices()[:n_cores]
    mesh = Mesh(np.asarray(devices), ("core",))
    in_specs = (PartitionSpec("core"),) * (n_params + n_outs)
    out_specs = (PartitionSpec("core"),) * len(out_names)
    sharded = jax.jit(
        shard_map(_body, mesh=mesh, in_specs=in_specs, out_specs=out_specs,
                  check_rep=False),
        donate_argnums=donate, keep_unused=True)

    def run(in_maps):
        gins = [np.concatenate([np.asarray(m[name]) for m in in_maps], axis=0)
                for name in in_names]
        gzeros = [np.concatenate([z] * n_cores, axis=0) for z in zero_outs]
        outs = sharded(*gins, *gzeros)
        res = []
        for c in range(n_cores):
            res.append({})
        for i, name in enumerate(out_names):
            arr = np.asarray(outs[i])
            per = arr.shape[0] // n_cores
            for c in range(n_cores):
                res[c][name] = arr[c * per:(c + 1) * per]
        return res

    _RUNNER_CACHE[key] = run
    return run


def _make_in_maps(inputs, wqkv_w, wqkv_b, wo_w):
    x = np.asarray(inputs, np.float32).reshape(B * V, S, E)
    wcache = {}
    in_maps = []
    xTb = {}
    for c in range(NCORE):
        g, hg = c // 2, c % 2
        if hg not in wcache:
            wcache[hg] = _host_weights(wqkv_w, wqkv_b, wo_w, hg)
        if g not in xTb:
            xTb[g] = _bf16(np.ascontiguousarray(x[g].T))
        wd = wcache[hg]
        in_maps.append(dict(
            xT=xTb[g], wqkvT=wd["wqkvT"], woT=wd["woT"],
            cs=wd["cs"], snS=wd["snS"], trib=wd["trib"],
            iden=wd["iden"]))
    return in_maps


def kernel(layer_idx=None, inputs=None, wqkv_w=None, wqkv_b=None,
           wo_w=None, wo_b=None):
    wqkv_w = np.asarray(wqkv_w, dtype=np.float32)
    wqkv_b = np.asarray(wqkv_b, dtype=np.float32)
    wo_w = np.asarray(wo_w, dtype=np.float32)
    wo_b = np.asarray(wo_b, dtype=np.float32)
    assert not np.any(wqkv_b), "nonzero wqkv_b not supported by this kernel build"

    nc = _get_nc()
    in_maps = _make_in_maps(inputs, wqkv_w, wqkv_b, wo_w)
    run = _get_runner(nc, NCORE)
    outs = run(in_maps)
    y = np.empty((B * V, S, E), dtype=np.float32)
    for g in range(B * V):
        acc = (outs[2 * g]["outT"].astype(np.float32)
               + outs[2 * g + 1]["outT"].astype(np.float32))   # (E, S)
        y[g] = acc.T
    y += wo_b[None, None, :]
    return y.reshape(B, V, S, E)


# revision 53
# speedup vs baseline: 1.0377x; 1.0377x over previous
"""Trainium2 Bass kernel for BaseMultiheadAttention (bf16, ~200us/core).

dims: B=1, V=4, S=2048, E=512, H=8, D=64 (head_dim), causal, interleaved RoPE.

Sharding (8 cores): core c -> bv index g = c//2, head-group hg = c%2
(4 heads each).  Each core computes its bv-slice's QKV projection restricted
to its 4 heads, RoPE, causal attention, and a partial output projection
(its heads' wO rows).  Host sums the two bf16 partials per bv index.

Design (evolved from the 283us fp32r baseline via NTFF profiling):
  - all matmul operands bf16: halves LDWEIGHTS time, DVE ops and DMA bytes
    (fp32r matmul column rate is already 1 col/cycle at >=256 moving cols,
    so the win is in weight loads / SBUF pressure, not column throughput)
  - the dominant limiter is the PE p-state governor (0.65/1.2/2.4 GHz; full
    clock only after ~3us of gapless execution), so everything is organized
    around keeping the tensor engine's queue non-empty:
      * ONE global software-pipelined step stream over all eight
        (head-stack, q-block) attention blocks: scores(step+2) issue ahead
        of PV(step), so PV never waits on the scalar-engine exp
      * stack-1 q/k projections+rope, softmax-normalize tails and the
        output projection are queued as single-thunk "fillers" drained one
        per step inside the attention stream (PE gap filler)
      * per-block PSUM PV accumulators are freed immediately via one DVE
        cast to SBUF (praw); 1/denom = Exp(-Ln(denom)) runs on the scalar
        engine ([1,1024] row, the ones-column of the 65-row PV lhsT
        accumulates denominators for free), and the broadcast ones-matmul +
        normalize multiplies are deferred fillers
  - causal diag masking by -1e30 psum prefill (identity x trib matmul)
    ahead of the scores accumulation - off the exp->PV dependency chain
  - PSUM map (8 banks): "big" [128,1024] x3 rotating (scores / proj /
    outproj / denom broadcast) + "po" [128,1024] x1 (PV accumulator)
  - input DMAs split by first-need across the sync/gpsimd/scalar queues
"""

import numpy as np

import concourse.bass as bass
import concourse.mybir as mybir
from concourse.tile import TileContext

# ---- problem dims (hardcoded per the task contract) ----
B, V, S, E, H = 1, 4, 2048, 512, 8
D = E // H            # 64
HG = 4                # heads per core
NCORE = 8
F32 = mybir.dt.float32
BF = mybir.dt.bfloat16


def _bf16(a):
    import ml_dtypes
    return np.asarray(a, np.float32).astype(ml_dtypes.bfloat16)


def _host_tables():
    pos = np.arange(S, dtype=np.float64)
    inv_freq = 1.0 / (10000.0 ** (np.arange(0, D, 2, dtype=np.float64) / D))
    freqs = pos[:, None] * inv_freq[None, :]          # (S, D/2)
    freqs = np.repeat(freqs, 2, axis=-1)              # (S, D) interleaved
    cosT = np.cos(freqs).T.astype(np.float32)         # (D, S)
    sinT = np.sin(freqs).T.astype(np.float32)
    cs = np.concatenate([cosT, cosT], axis=0)         # (128, S) two-head stack
    sn = np.concatenate([sinT, sinT], axis=0)
    sgn = np.tile(np.array([-1.0, 1.0], np.float32), D // 2)[:, None]
    snS = sn * np.concatenate([sgn, sgn], axis=0)
    # -inf prefill for the 128-wide diag block (rows=k, cols=q): mask q < k,
    # padded to 512 with zeros (the strictly-lower remainder of the stream)
    trib = np.zeros((128, 512), np.float32)
    for p in range(128):
        trib[p, :p] = -1.0e30
    iden = np.eye(128, dtype=np.float32)
    return cs, snS, trib, iden


def _host_weights(wqkv_w, wqkv_b, wo_w, hg):
    """Per-head-group weight slices in the kernel's layouts (bf16)."""
    heads = [hg * HG + h for h in range(HG)]
    cs, snS, trib, iden = _host_tables()
    # feature index inside each qkv block: d*H + h
    def rows(block, h):
        d = np.arange(D)
        return block * E + d * H + h
    def to_T(Wh):   # (HG, D, E) -> (E, HG*D) with col = h*64+d
        return np.transpose(Wh, (2, 0, 1)).reshape(E, HG * D).astype(np.float32)
    Wq = np.stack([wqkv_w[rows(0, h)] for h in heads])   # (HG, D, E)
    Wk = np.stack([wqkv_w[rows(1, h)] for h in heads])
    Wv = np.stack([wqkv_w[rows(2, h)] for h in heads])
    wqkvT = np.concatenate([to_T(Wq), to_T(Wk), to_T(Wv)], axis=1)  # (E, 768)
    # wo rows for this head group: out feature = h_global*64 + d
    woT = np.stack([wo_w[:, (hg * HG + h) * D:(hg * HG + h + 1) * D].T
                    for h in range(HG)])                 # (HG, D, E)
    woT = woT.reshape(HG * D, E).astype(np.float32)      # (256, 512)
    return dict(wqkvT=_bf16(wqkvT), woT=_bf16(woT),
                cs=_bf16(cs), snS=_bf16(snS), trib=_bf16(trib),
                iden=_bf16(iden))


_MAX_WAITS = {"Matmult": 1}          # per-opcode cap; default below
_DEF_MAX_WAITS = 1


def _split_excess_waits(nc):
    """This walrus build encodes at most ~1 sync-wait per instruction.
    Post-process the serialized BIR: hoist excess on_wait entries onto
    same-engine NoOp carriers emitted immediately before the instruction."""
    import orjson

    orig = nc.to_json_bytes

    def patched(_self=None):
        d = orjson.loads(orig())
        for fn in d.get("functions", []):
            for bb in fn.get("basicblocks", fn.get("blocks", [])):
                insts = bb.get("instructions")
                if insts is None:
                    continue
                out, nctr = [], 0
                for inst in insts:
                    si = inst.get("sync_info")
                    waits = (si or {}).get("on_wait") or []
                    cap = _MAX_WAITS.get(inst.get("opcode"), _DEF_MAX_WAITS)
                    if len(waits) > cap:
                        keep = waits[:cap]
                        extra = waits[cap:]
                        for w in extra:
                            nctr += 1
                            out.append({
                                "debug": inst.get("debug", 0),
                                "engine": inst["engine"],
                                "ins": [], "outs": [],
                                "name": f"{inst['name']}_w{nctr}",
                                "opcode": "NoOp",
                                "sync_info": {"on_wait": [w],
                                              "on_update": []},
                            })
                        si["on_wait"] = keep
                    out.append(inst)
                bb["instructions"] = out
        return orjson.dumps(d)

    nc.to_json_bytes = patched
    return nc


def build_nc(reps=1, debug=False):
    nc = bass.Bass()
    xT = nc.declare_dram_parameter("xT", [E, S], BF, isOutput=False)
    wqkvT = nc.declare_dram_parameter("wqkvT", [E, 3 * HG * D], BF,
                                      isOutput=False)
    woT = nc.declare_dram_parameter("woT", [HG * D, E], BF, isOutput=False)
    cs = nc.declare_dram_parameter("cs", [128, S], BF, isOutput=False)
    snS = nc.declare_dram_parameter("snS", [128, S], BF, isOutput=False)
    trib = nc.declare_dram_parameter("trib", [128, 512], BF, isOutput=False)
    iden = nc.declare_dram_parameter("iden", [128, 128], BF, isOutput=False)
    outT = nc.declare_dram_parameter("outT", [E, S], BF, isOutput=True)
    if debug:
        dbg_qrot = nc.declare_dram_parameter("dbg_qrot", [128, S], BF,
                                             isOutput=True)
        dbg_krot = nc.declare_dram_parameter("dbg_krot", [128, S], BF,
                                             isOutput=True)
        dbg_vt = nc.declare_dram_parameter("dbg_vt", [128, HG * 65], BF,
                                           isOutput=True)
        dbg_on = nc.declare_dram_parameter("dbg_on", [128, 512], BF,
                                           isOutput=True)

    SWAP = [1, 0, 3, 2, 5, 4, 7, 6, 9, 8, 11, 10, 13, 12, 15, 14,
            17, 16, 19, 18, 21, 20, 23, 22, 25, 24, 27, 26, 29, 28, 31, 30]
    scale = 1.0 / np.sqrt(D)

    with TileContext(nc) as tc:
      for _rep in range(reps):
        with (
            tc.tile_pool(name="const", bufs=1) as cpool,
            tc.tile_pool(name="qk", bufs=1) as qkpool,
            tc.tile_pool(name="v", bufs=1) as vpool,
            tc.tile_pool(name="pt", bufs=3) as ptpool,
            tc.tile_pool(name="rope", bufs=3) as rpool,
            tc.tile_pool(name="on", bufs=1) as onpool,
            tc.tile_pool(name="sums", bufs=2) as spool,
            tc.tile_pool(name="oc", bufs=3) as ocpool,
        ):
            # ---- input DMAs, first-needed-first, spread over the three
            # DMA-capable queues (sync / gpsimd / scalar) ----
            xt = [cpool.tile([128, S], BF, tag=f"xt{e4}", name=f"xt{e4}")
                  for e4 in range(4)]
            w_t = [cpool.tile([128, 3 * HG * D], BF, tag=f"w{e4}",
                              name=f"w{e4}") for e4 in range(4)]
            cs_t = cpool.tile([128, S], BF, tag="cs", name="cs_t")
            sn_t = cpool.tile([128, S], BF, tag="sn", name="sn_t")
            trib_t = cpool.tile([128, 512], BF, tag="trib", name="trib_t")
            iden_t = cpool.tile([128, 128], BF, tag="iden", name="iden_t")
            wo_t = [cpool.tile([128, E], BF, tag=f"wo{f2}", name=f"wo{f2}")
                    for f2 in range(2)]
            # k-weight chunks (first projection) then rope tables
            for e4 in range(4):
                nc.scalar.dma_start(
                    w_t[e4][:, HG * D:2 * HG * D],
                    wqkvT[e4 * 128:(e4 + 1) * 128, HG * D:2 * HG * D])
            # x chunks, block-interleaved so early s-blocks land first
            for blk in range(4):
                for e4 in range(4):
                    eng = nc.sync if (e4 % 2 == 0) else nc.gpsimd
                    eng.dma_start(
                        xt[e4][:, blk * 512:(blk + 1) * 512],
                        xT[e4 * 128:(e4 + 1) * 128,
                           blk * 512:(blk + 1) * 512])
            nc.scalar.dma_start(cs_t[:, :], cs[:, :])
            nc.scalar.dma_start(sn_t[:, :], snS[:, :])
            for e4 in range(4):   # q-weight chunks
                nc.scalar.dma_start(
                    w_t[e4][:, 0:HG * D],
                    wqkvT[e4 * 128:(e4 + 1) * 128, 0:HG * D])
            for e4 in range(4):   # v-weight chunks
                nc.scalar.dma_start(
                    w_t[e4][:, 2 * HG * D:3 * HG * D],
                    wqkvT[e4 * 128:(e4 + 1) * 128, 2 * HG * D:3 * HG * D])
            nc.scalar.dma_start(trib_t[:, :], trib[:, :])
            nc.scalar.dma_start(iden_t[:, :], iden[:, :])
            for f2 in range(2):
                nc.scalar.dma_start(wo_t[f2][:, :],
                                    woT[f2 * 128:(f2 + 1) * 128, :])
            ones_b = cpool.tile([1, D], BF, tag="ones", name="ones_b")
            nc.vector.memset(ones_b[:, :], 1.0)

            qrot, krot = {}, {}
            onorm = {}
            v_t = []
            with tc.tile_pool(name="psA", bufs=1, space="PSUM") as psA:
                # PSUM bank budget (8 banks of [128,512]f32):
                #   duo  [128,1024] x2 bufs = 4 banks (scores)
                #   po   [128,1024] x1 buf  = 2 banks (PV accum, both heads)
                #   sm   [128,512]  x2 bufs = 2 banks (qkv proj, rope, outproj)

                filler = []     # queued single-matmul thunks (PE gap filler)

                def drain_filler(n=1):
                    for _ in range(n):
                        if filler:
                            filler.pop(0)()

                def project_rope(tgt, base, dst, st, queued=False):
                    """Q/K projection + rotary for stack st (2 heads)."""
                    rt = qkpool.tile([128, S], BF, tag=f"{tgt}rot{st}",
                                     name=f"{tgt}rot{st}")
                    dst[st] = rt
                    fcol = base + st * 2 * D

                    def mk(blk):
                        def f():
                            s0 = blk * 512
                            pq = psA.tile([128, 1024], F32, tag="big",
                                          bufs=3, name="pq")
                            for e4 in range(4):
                                nc.tensor.matmul(
                                    pq[:, 0:512],
                                    w_t[e4][:, fcol:fcol + 128],
                                    xt[e4][:, s0:s0 + 512],
                                    start=(e4 == 0), stop=(e4 == 3))
                            sh_t = rpool.tile([128, 512], F32, tag="ropesh",
                                              name="ropesh")
                            t1 = rpool.tile([128, 512], BF, tag="ropet1",
                                            name="ropet1")
                            t2 = rpool.tile([128, 512], BF, tag="ropet2",
                                            name="ropet2")
                            nc.vector.stream_shuffle(sh_t[:, :], pq[:, 0:512],
                                                     SWAP)
                            nc.vector.tensor_mul(
                                t1[:, :], pq[:, 0:512], cs_t[:, s0:s0 + 512])
                            nc.vector.tensor_mul(
                                t2[:, :], sh_t[:, :], sn_t[:, s0:s0 + 512])
                            nc.vector.tensor_add(
                                rt[:, s0:s0 + 512], t1[:, :], t2[:, :])
                        return f

                    for blk in range(4):
                        f = mk(blk)
                        if queued:
                            filler.append(f)
                        else:
                            f()

                def project_v():
                    for i in range(S // 128):
                        vt = vpool.tile([128, HG * 65], BF, tag=f"v{i}",
                                        name=f"v{i}")
                        v_t.append(vt)
                        pvb = psA.tile([128, 1024], F32, tag="big", bufs=3,
                                       name="pvb")
                        pv = pvb
                        for e4 in range(4):
                            nc.tensor.matmul(
                                pv[:, 0:HG * D],
                                xt[e4][:, i * 128:(i + 1) * 128],
                                w_t[e4][:, 2 * HG * D:3 * HG * D],
                                start=(e4 == 0), stop=(e4 == 3))
                        vt3 = vt[:, :].rearrange("p (h x) -> p h x", h=HG)
                        nc.vector.memset(vt3[:, :, D:D + 1], 1.0)
                        nc.vector.tensor_copy(
                            vt3[:, :, 0:D],
                            pv[:, 0:HG * D].rearrange("p (h d) -> p h d",
                                                      h=HG))

                def emit_scores(hp, j, i):
                    """Scores + exp for k-tile i of q-block j.  Diag tiles
                    are causal-masked by prefilling the psum with -1e30
                    above the diagonal (identity matmul on trib)."""
                    r = i - 4 * j
                    offs = 128 * r if r >= 0 else 0
                    sc = psA.tile([128, 1024], F32, tag="big", bufs=3,
                                  name="sc")
                    pt = ptpool.tile([128, 1024], BF, tag="pt", bufs=5,
                                     name="pt")
                    for hh in range(2):
                        nc.tensor.matmul(
                            sc[:, hh * 512 + offs:hh * 512 + 512],
                            krot[hp][hh * D:hh * D + D,
                                     i * 128:(i + 1) * 128],
                            qrot[hp][hh * D:hh * D + D,
                                     j * 512 + offs:(j + 1) * 512],
                            start=True, stop=(r < 0))
                        if r >= 0:
                            nc.tensor.matmul(
                                sc[:, hh * 512 + offs:hh * 512 + offs + 128],
                                iden_t[:, :], trib_t[:, 0:128],
                                start=False, stop=True)
                    src = sc[:, :].rearrange("p (h x) -> p h x",
                                             h=2)[:, :, offs:512]
                    dstv = pt[:, :].rearrange("p (h x) -> p h x",
                                              h=2)[:, :, offs:512]
                    nc.scalar.activation(
                        dstv, src, mybir.ActivationFunctionType.Exp,
                        scale=float(scale))
                    return pt, offs

                def emit_pv(hp, j, i, pt, offs, n_i):
                    if i == 0:
                        po_cur[0] = psA.tile([128, 1024], F32, tag="po",
                                             bufs=1, name="po")
                    po = po_cur[0]
                    for hh in range(2):
                        h = 2 * hp + hh
                        nc.tensor.matmul(
                            po[0:65, hh * 512 + offs:hh * 512 + 512],
                            v_t[i][:, h * 65:h * 65 + 65],
                            pt[:, hh * 512 + offs:hh * 512 + 512],
                            start=(i == 0), stop=(i == n_i - 1))
                    if i == n_i - 1:
                        finish_block(hp, j, po)

                po_cur = [None]

                def queue_outproj(j):
                    """Push outproj(j) as 4 filler thunks (one per e-chunk)."""
                    def mk(j, eb):
                        def f():
                            pp = psA.tile([128, 1024], F32, tag="big",
                                          bufs=3, name="pp")
                            for f2 in range(2):
                                nc.tensor.matmul(
                                    pp[:, 0:512],
                                    wo_t[f2][:, eb * 128:(eb + 1) * 128],
                                    onorm[(j, f2)][:, :],
                                    start=(f2 == 0), stop=(f2 == 1))
                            oc = ocpool.tile([128, 512], BF,
                                             tag="oc", name="oc")
                            nc.vector.tensor_copy(oc[:, :], pp[:, 0:512])
                            nc.sync.dma_start(
                                outT[eb * 128:(eb + 1) * 128,
                                     j * 512:(j + 1) * 512],
                                oc[:, :])
                        return f
                    for eb in range(4):
                        filler.append(mk(j, eb))

                def queue_norm_tail(j, hp, praw, rrow_b):
                    """Deferred: broadcast 1/denom via ones-matmul into a
                    fresh duo-psum tile, multiply praw by it into onorm."""
                    def f():
                        onj = onpool.tile([128, 512], BF, tag=f"on{j}{hp}",
                                          name=f"on{j}{hp}")
                        onorm[(j, hp)] = onj
                        bc = psA.tile([128, 1024], F32, tag="big", bufs=3,
                                      name="bc")
                        for hh in range(2):
                            nc.tensor.matmul(
                                bc[0:D, hh * 512:hh * 512 + 512],
                                ones_b[:, :],
                                rrow_b[:, hh * 512:hh * 512 + 512],
                                start=True, stop=True)
                        for hh in range(2):
                            nc.vector.tensor_mul(
                                onj[hh * D:hh * D + D, :],
                                praw[0:D, hh * 512:hh * 512 + 512],
                                bc[0:D, hh * 512:hh * 512 + 512])
                        if debug and j == 1 and hp == 0:
                            nc.sync.dma_start(dbg_on[:, :], onj[:, :])
                        if hp == 1:
                            queue_outproj(j)
                    filler.append(f)

                def finish_block(hp, j, po):
                    # free po fast: copy raw PV (rows 0:64 per head + denom
                    # row 64) to SBUF, compute 1/denom = Exp(-Ln(denom)) on
                    # the scalar engine; the broadcast + normalize multiply
                    # are deferred into the filler queue
                    praw = spool.tile([65, 1024], BF, tag=f"praw{j}{hp}",
                                      name="praw")
                    nc.vector.tensor_copy(praw[:, :], po[0:65, :])
                    rln = spool.tile([1, 1024], F32, tag="rln", name="rln")
                    rrow_b = spool.tile([1, 1024], BF, tag=f"rrowb{j}{hp}",
                                        name="rrowb")
                    nc.scalar.activation(rln[:, :], praw[64:65, :],
                                         mybir.ActivationFunctionType.Ln)
                    nc.scalar.activation(rrow_b[:, :], rln[:, :],
                                         mybir.ActivationFunctionType.Exp,
                                         scale=-1.0)
                    queue_norm_tail(j, hp, praw, rrow_b)

                # schedule: stack-1 projections, normalize tails and outproj
                # run as fillers inside one GLOBAL software-pipelined step
                # stream (scores lookahead 2, PV behind) spanning all eight
                # (stack, q-block) attention blocks, so the PE never drains
                project_rope("k", HG * D, krot, 0)
                project_rope("q", 0, qrot, 0)
                project_v()
                project_rope("k", HG * D, krot, 1, queued=True)
                project_rope("q", 0, qrot, 1, queued=True)
                SCHED = ((0, 1), (0, 2), (0, 3), (1, 1), (0, 0),
                         (1, 2), (1, 0), (1, 3))
                steps = [(hp, j, i) for hp, j in SCHED
                         for i in range(4 * j + 4)]
                pts = {}
                for idx in range(len(steps)):
                    if idx == 0:
                        pts[steps[0]] = emit_scores(*steps[0])
                        pts[steps[1]] = emit_scores(*steps[1])
                    if idx + 2 < len(steps):
                        pts[steps[idx + 2]] = emit_scores(*steps[idx + 2])
                    drain_filler()
                    hp, j, i = steps[idx]
                    pt, offs = pts.pop((hp, j, i))
                    emit_pv(hp, j, i, pt, offs, 4 * j + 4)
                while filler:
                    filler.pop(0)()
                if debug:
                    nc.sync.dma_start(dbg_qrot[:, :], qrot[0][:, :])
                    nc.sync.dma_start(dbg_krot[:, :], krot[0][:, :])
                    nc.sync.dma_start(dbg_vt[:, :], v_t[0][:, :])
    return _split_excess_waits(nc)


_NC_CACHE = {}


def _get_nc(reps=1):
    if reps not in _NC_CACHE:
        _NC_CACHE[reps] = build_nc(reps)
    return _NC_CACHE[reps]


_RUNNER_CACHE = {}


def _get_runner(nc, n_cores):
    """Clone of bass2jax.run_bass_via_pjrt's multi-core path with the
    jitted callable cached so repeat calls skip retracing."""
    key = id(nc)
    if key in _RUNNER_CACHE:
        return _RUNNER_CACHE[key]
    import jax
    from jax.sharding import Mesh, PartitionSpec
    from jax.experimental.shard_map import shard_map
    from concourse import bass2jax as b2j

    b2j.install_neuronx_cc_hook()
    partition_name = (nc.partition_id_tensor.name
                      if nc.partition_id_tensor else None)
    in_names, out_names, out_avals, zero_outs = [], [], [], []
    for alloc in nc.m.functions[0].allocations:
        if not isinstance(alloc, mybir.MemoryLocationSet):
            continue
        name = alloc.memorylocations[0].name
        if alloc.kind == "ExternalInput":
            if name != partition_name:
                in_names.append(name)
        elif alloc.kind == "ExternalOutput":
            shape = tuple(alloc.tensor_shape)
            dtype = mybir.dt.np(alloc.dtype)
            out_names.append(name)
            out_avals.append(jax.core.ShapedArray(shape, dtype))
            zero_outs.append(np.zeros(shape, dtype))
    n_params = len(in_names)
    n_outs = len(out_avals)
    in_names_all = list(in_names) + list(out_names)
    if partition_name is not None:
        in_names_all.append(partition_name)
    donate = tuple(range(n_params, n_params + n_outs))

    def _body(*args):
        operands = list(args)
        if partition_name is not None:
            operands.append(b2j.partition_id_tensor())
        outs = b2j._bass_exec_p.bind(
            *operands,
            out_avals=tuple(out_avals),
            in_names=tuple(in_names_all),
            out_names=tuple(out_names),
            lowering_input_output_aliases=(),
            sim_require_finite=True,
            sim_require_nnan=True,
            nc=nc,
        )
        return tuple(outs)

    devices = jax.devices()[:n_cores]
    mesh = Mesh(np.asarray(devices), ("core",))
    in_specs = (PartitionSpec("core"),) * (n_params + n_outs)
    out_specs = (PartitionSpec("core"),) * len(out_names)
    sharded = jax.jit(
        shard_map(_body, mesh=mesh, in_specs=in_specs, out_specs=out_specs,
                  check_rep=False),
        donate_argnums=donate, keep_unused=True)

    def run(in_maps):
        gins = [np.concatenate([np.asarray(m[name]) for m in in_maps], axis=0)
                for name in in_names]
        gzeros = [np.concatenate([z] * n_cores, axis=0) for z in zero_outs]
        outs = sharded(*gins, *gzeros)
        res = []
        for c in range(n_cores):
            res.append({})
        for i, name in enumerate(out_names):
            arr = np.asarray(outs[i])
            per = arr.shape[0] // n_cores
            for c in range(n_cores):
                res[c][name] = arr[c * per:(c + 1) * per]
        return res

    _RUNNER_CACHE[key] = run
    return run


def _make_in_maps(inputs, wqkv_w, wqkv_b, wo_w):
    x = np.asarray(inputs, np.float32).reshape(B * V, S, E)
    wcache = {}
    in_maps = []
    xTb = {}
    for c in range(NCORE):
        g, hg = c // 2, c % 2
        if hg not in wcache:
            wcache[hg] = _host_weights(wqkv_w, wqkv_b, wo_w, hg)
        if g not in xTb:
            xTb[g] = _bf16(np.ascontiguousarray(x[g].T))
        wd = wcache[hg]
        in_maps.append(dict(
            xT=xTb[g], wqkvT=wd["wqkvT"], woT=wd["woT"],
            cs=wd["cs"], snS=wd["snS"], trib=wd["trib"],
            iden=wd["iden"]))
    return in_maps


def kernel(layer_idx=None, inputs=None, wqkv_w=None, wqkv_b=None,
           wo_w=None, wo_b=None):
    wqkv_w = np.asarray(wqkv_w, dtype=np.float32)
    wqkv_b = np.asarray(wqkv_b, dtype=np.float32)
    wo_w = np.asarray(wo_w, dtype=np.float32)
    wo_b = np.asarray(wo_b, dtype=np.float32)
    assert not np.any(wqkv_b), "nonzero wqkv_b not supported by this kernel build"

    nc = _get_nc()
    in_maps = _make_in_maps(inputs, wqkv_w, wqkv_b, wo_w)
    run = _get_runner(nc, NCORE)
    outs = run(in_maps)
    y = np.empty((B * V, S, E), dtype=np.float32)
    for g in range(B * V):
        acc = (outs[2 * g]["outT"].astype(np.float32)
               + outs[2 * g + 1]["outT"].astype(np.float32))   # (E, S)
        y[g] = acc.T
    y += wo_b[None, None, :]
    return y.reshape(B, V, S, E)


# revision 54
# speedup vs baseline: 1.2981x; 1.2510x over previous
"""Trainium2 Bass kernel for BaseMultiheadAttention (bf16, ~200us/core).

dims: B=1, V=4, S=2048, E=512, H=8, D=64 (head_dim), causal, interleaved RoPE.

Sharding (8 cores): core c -> bv index g = c//2, head-group hg = c%2
(4 heads each).  Each core computes its bv-slice's QKV projection restricted
to its 4 heads, RoPE, causal attention, and a partial output projection
(its heads' wO rows).  Host sums the two bf16 partials per bv index.

Design (evolved from the 283us fp32r baseline via NTFF profiling):
  - all matmul operands bf16: halves LDWEIGHTS time, DVE ops and DMA bytes
    (fp32r matmul column rate is already 1 col/cycle at >=256 moving cols,
    so the win is in weight loads / SBUF pressure, not column throughput)
  - the dominant limiter is the PE p-state governor (0.65/1.2/2.4 GHz; full
    clock only after ~3us of gapless execution), so everything is organized
    around keeping the tensor engine's queue non-empty:
      * ONE global software-pipelined step stream over all eight
        (head-stack, q-block) attention blocks: scores(step+2) issue ahead
        of PV(step), so PV never waits on the scalar-engine exp
      * stack-1 q/k projections+rope, softmax-normalize tails and the
        output projection are queued as single-thunk "fillers" drained one
        per step inside the attention stream (PE gap filler)
      * per-block PSUM PV accumulators are freed immediately via one DVE
        cast to SBUF (praw); 1/denom = Exp(-Ln(denom)) runs on the scalar
        engine ([1,1024] row, the ones-column of the 65-row PV lhsT
        accumulates denominators for free), and the broadcast ones-matmul +
        normalize multiplies are deferred fillers
  - causal diag masking by -1e30 psum prefill (identity x trib matmul)
    ahead of the scores accumulation - off the exp->PV dependency chain
  - PSUM map (8 banks): "big" [128,1024] x3 rotating (scores / proj /
    outproj / denom broadcast) + "po" [128,1024] x1 (PV accumulator)
  - input DMAs split by first-need across the sync/gpsimd/scalar queues
"""

import numpy as np

import concourse.bass as bass
import concourse.mybir as mybir
from concourse.tile import TileContext

# ---- problem dims (hardcoded per the task contract) ----
B, V, S, E, H = 1, 4, 2048, 512, 8
D = E // H            # 64
HG = 4                # heads per core
NCORE = 8
F32 = mybir.dt.float32
BF = mybir.dt.bfloat16


def _bf16(a):
    import ml_dtypes
    return np.asarray(a, np.float32).astype(ml_dtypes.bfloat16)


def _host_tables():
    pos = np.arange(S, dtype=np.float64)
    inv_freq = 1.0 / (10000.0 ** (np.arange(0, D, 2, dtype=np.float64) / D))
    freqs = pos[:, None] * inv_freq[None, :]          # (S, D/2)
    freqs = np.repeat(freqs, 2, axis=-1)              # (S, D) interleaved
    cosT = np.cos(freqs).T.astype(np.float32)         # (D, S)
    sinT = np.sin(freqs).T.astype(np.float32)
    cs = np.concatenate([cosT, cosT], axis=0)         # (128, S) two-head stack
    sn = np.concatenate([sinT, sinT], axis=0)
    sgn = np.tile(np.array([-1.0, 1.0], np.float32), D // 2)[:, None]
    snS = sn * np.concatenate([sgn, sgn], axis=0)
    # -inf prefill for the 128-wide diag block (rows=k, cols=q): mask q < k,
    # padded to 512 with zeros (the strictly-lower remainder of the stream)
    trib = np.zeros((128, 512), np.float32)
    for p in range(128):
        trib[p, :p] = -1.0e30
    iden = np.eye(128, dtype=np.float32)
    return cs, snS, trib, iden


def _host_weights(wqkv_w, wqkv_b, wo_w, hg):
    """Per-head-group weight slices in the kernel's layouts (bf16)."""
    heads = [hg * HG + h for h in range(HG)]
    cs, snS, trib, iden = _host_tables()
    # feature index inside each qkv block: d*H + h
    def rows(block, h):
        d = np.arange(D)
        return block * E + d * H + h
    def to_T(Wh):   # (HG, D, E) -> (E, HG*D) with col = h*64+d
        return np.transpose(Wh, (2, 0, 1)).reshape(E, HG * D).astype(np.float32)
    Wq = np.stack([wqkv_w[rows(0, h)] for h in heads])   # (HG, D, E)
    Wk = np.stack([wqkv_w[rows(1, h)] for h in heads])
    Wv = np.stack([wqkv_w[rows(2, h)] for h in heads])
    wqkvT = np.concatenate([to_T(Wq), to_T(Wk), to_T(Wv)], axis=1)  # (E, 768)
    # wo rows for this head group: out feature = h_global*64 + d
    woT = np.stack([wo_w[:, (hg * HG + h) * D:(hg * HG + h + 1) * D].T
                    for h in range(HG)])                 # (HG, D, E)
    woT = woT.reshape(HG * D, E).astype(np.float32)      # (256, 512)
    return dict(wqkvT=_bf16(wqkvT), woT=_bf16(woT),
                cs=_bf16(cs), snS=_bf16(snS), trib=_bf16(trib),
                iden=_bf16(iden))


_MAX_WAITS = {"Matmult": 1}          # per-opcode cap; default below
_DEF_MAX_WAITS = 1


def _split_excess_waits(nc):
    """This walrus build encodes at most ~1 sync-wait per instruction.
    Post-process the serialized BIR: hoist excess on_wait entries onto
    same-engine NoOp carriers emitted immediately before the instruction."""
    import orjson

    orig = nc.to_json_bytes

    def patched(_self=None):
        d = orjson.loads(orig())
        for fn in d.get("functions", []):
            for bb in fn.get("basicblocks", fn.get("blocks", [])):
                insts = bb.get("instructions")
                if insts is None:
                    continue
                out, nctr = [], 0
                for inst in insts:
                    si = inst.get("sync_info")
                    waits = (si or {}).get("on_wait") or []
                    cap = _MAX_WAITS.get(inst.get("opcode"), _DEF_MAX_WAITS)
                    if len(waits) > cap:
                        keep = waits[:cap]
                        extra = waits[cap:]
                        for w in extra:
                            nctr += 1
                            out.append({
                                "debug": inst.get("debug", 0),
                                "engine": inst["engine"],
                                "ins": [], "outs": [],
                                "name": f"{inst['name']}_w{nctr}",
                                "opcode": "NoOp",
                                "sync_info": {"on_wait": [w],
                                              "on_update": []},
                            })
                        si["on_wait"] = keep
                    out.append(inst)
                bb["instructions"] = out
        return orjson.dumps(d)

    nc.to_json_bytes = patched
    return nc


def build_nc(reps=1, debug=False):
    nc = bass.Bass()
    xT = nc.declare_dram_parameter("xT", [E, S], BF, isOutput=False)
    wqkvT = nc.declare_dram_parameter("wqkvT", [E, 3 * HG * D], BF,
                                      isOutput=False)
    woT = nc.declare_dram_parameter("woT", [HG * D, E], BF, isOutput=False)
    cs = nc.declare_dram_parameter("cs", [128, S], BF, isOutput=False)
    snS = nc.declare_dram_parameter("snS", [128, S], BF, isOutput=False)
    trib = nc.declare_dram_parameter("trib", [128, 512], BF, isOutput=False)
    iden = nc.declare_dram_parameter("iden", [128, 128], BF, isOutput=False)
    outT = nc.declare_dram_parameter("outT", [E, S], BF, isOutput=True)
    if debug:
        dbg_qrot = nc.declare_dram_parameter("dbg_qrot", [128, S], BF,
                                             isOutput=True)
        dbg_krot = nc.declare_dram_parameter("dbg_krot", [128, S], BF,
                                             isOutput=True)
        dbg_vt = nc.declare_dram_parameter("dbg_vt", [128, HG * 65], BF,
                                           isOutput=True)
        dbg_on = nc.declare_dram_parameter("dbg_on", [128, 512], BF,
                                           isOutput=True)

    SWAP = [1, 0, 3, 2, 5, 4, 7, 6, 9, 8, 11, 10, 13, 12, 15, 14,
            17, 16, 19, 18, 21, 20, 23, 22, 25, 24, 27, 26, 29, 28, 31, 30]
    scale = 1.0 / np.sqrt(D)

    with TileContext(nc) as tc:
      for _rep in range(reps):
        with (
            tc.tile_pool(name="const", bufs=1) as cpool,
            tc.tile_pool(name="qk", bufs=1) as qkpool,
            tc.tile_pool(name="v", bufs=1) as vpool,
            tc.tile_pool(name="pt", bufs=3) as ptpool,
            tc.tile_pool(name="rope", bufs=3) as rpool,
            tc.tile_pool(name="on", bufs=1) as onpool,
            tc.tile_pool(name="sums", bufs=2) as spool,
            tc.tile_pool(name="oc", bufs=3) as ocpool,
        ):
            # ---- input DMAs, first-needed-first, spread over the three
            # DMA-capable queues (sync / gpsimd / scalar) ----
            xt = [cpool.tile([128, S], BF, tag=f"xt{e4}", name=f"xt{e4}")
                  for e4 in range(4)]
            w_t = [cpool.tile([128, 3 * HG * D], BF, tag=f"w{e4}",
                              name=f"w{e4}") for e4 in range(4)]
            cs_t = cpool.tile([128, S], BF, tag="cs", name="cs_t")
            sn_t = cpool.tile([128, S], BF, tag="sn", name="sn_t")
            trib_t = cpool.tile([128, 512], BF, tag="trib", name="trib_t")
            iden_t = cpool.tile([128, 128], BF, tag="iden", name="iden_t")
            wo_t = [cpool.tile([128, E], BF, tag=f"wo{f2}", name=f"wo{f2}")
                    for f2 in range(2)]
            # k-weight chunks (first projection) then rope tables
            for e4 in range(4):
                nc.scalar.dma_start(
                    w_t[e4][:, HG * D:2 * HG * D],
                    wqkvT[e4 * 128:(e4 + 1) * 128, HG * D:2 * HG * D])
            # x chunks, block-interleaved so early s-blocks land first
            for blk in range(4):
                for e4 in range(4):
                    eng = nc.sync if (e4 % 2 == 0) else nc.gpsimd
                    eng.dma_start(
                        xt[e4][:, blk * 512:(blk + 1) * 512],
                        xT[e4 * 128:(e4 + 1) * 128,
                           blk * 512:(blk + 1) * 512])
            nc.scalar.dma_start(cs_t[:, :], cs[:, :])
            nc.scalar.dma_start(sn_t[:, :], snS[:, :])
            for e4 in range(4):   # q-weight chunks
                nc.scalar.dma_start(
                    w_t[e4][:, 0:HG * D],
                    wqkvT[e4 * 128:(e4 + 1) * 128, 0:HG * D])
            for e4 in range(4):   # v-weight chunks
                nc.scalar.dma_start(
                    w_t[e4][:, 2 * HG * D:3 * HG * D],
                    wqkvT[e4 * 128:(e4 + 1) * 128, 2 * HG * D:3 * HG * D])
            nc.scalar.dma_start(trib_t[:, :], trib[:, :])
            nc.scalar.dma_start(iden_t[:, :], iden[:, :])
            for f2 in range(2):
                nc.scalar.dma_start(wo_t[f2][:, :],
                                    woT[f2 * 128:(f2 + 1) * 128, :])
            ones_b = cpool.tile([1, D], BF, tag="ones", name="ones_b")
            nc.vector.memset(ones_b[:, :], 1.0)

            qrot, krot = {}, {}
            onorm = {}
            v_t = []
            with tc.tile_pool(name="psA", bufs=1, space="PSUM") as psA:
                # PSUM bank budget (8 banks of [128,512]f32):
                #   duo  [128,1024] x2 bufs = 4 banks (scores)
                #   po   [128,1024] x1 buf  = 2 banks (PV accum, both heads)
                #   sm   [128,512]  x2 bufs = 2 banks (qkv proj, rope, outproj)

                filler = []     # queued single-matmul thunks (PE gap filler)

                def drain_filler(n=1):
                    for _ in range(n):
                        if filler:
                            filler.pop(0)()

                def project_rope(tgt, base, dst, st, queued=False):
                    """Q/K projection + rotary for stack st (2 heads)."""
                    rt = qkpool.tile([128, S], BF, tag=f"{tgt}rot{st}",
                                     name=f"{tgt}rot{st}")
                    dst[st] = rt
                    fcol = base + st * 2 * D

                    def mk(blk):
                        def f():
                            s0 = blk * 512
                            pq = psA.tile([128, 1024], F32, tag="big",
                                          bufs=3, name="pq")
                            for e4 in range(4):
                                nc.tensor.matmul(
                                    pq[:, 0:512],
                                    w_t[e4][:, fcol:fcol + 128],
                                    xt[e4][:, s0:s0 + 512],
                                    start=(e4 == 0), stop=(e4 == 3))
                            sh_t = rpool.tile([128, 512], F32, tag="ropesh",
                                              name="ropesh")
                            t1 = rpool.tile([128, 512], BF, tag="ropet1",
                                            name="ropet1")
                            t2 = rpool.tile([128, 512], BF, tag="ropet2",
                                            name="ropet2")
                            nc.vector.stream_shuffle(sh_t[:, :], pq[:, 0:512],
                                                     SWAP)
                            nc.vector.tensor_mul(
                                t1[:, :], pq[:, 0:512], cs_t[:, s0:s0 + 512])
                            nc.vector.tensor_mul(
                                t2[:, :], sh_t[:, :], sn_t[:, s0:s0 + 512])
                            nc.vector.tensor_add(
                                rt[:, s0:s0 + 512], t1[:, :], t2[:, :])
                        return f

                    for blk in range(4):
                        f = mk(blk)
                        if queued:
                            filler.append(f)
                        else:
                            f()

                def project_v():
                    for i in range(S // 128):
                        vt = vpool.tile([128, HG * 65], BF, tag=f"v{i}",
                                        name=f"v{i}")
                        v_t.append(vt)
                        pvb = psA.tile([128, 1024], F32, tag="big", bufs=3,
                                       name="pvb")
                        pv = pvb
                        for e4 in range(4):
                            nc.tensor.matmul(
                                pv[:, 0:HG * D],
                                xt[e4][:, i * 128:(i + 1) * 128],
                                w_t[e4][:, 2 * HG * D:3 * HG * D],
                                start=(e4 == 0), stop=(e4 == 3))
                        vt3 = vt[:, :].rearrange("p (h x) -> p h x", h=HG)
                        nc.vector.memset(vt3[:, :, D:D + 1], 1.0)
                        nc.vector.tensor_copy(
                            vt3[:, :, 0:D],
                            pv[:, 0:HG * D].rearrange("p (h d) -> p h d",
                                                      h=HG))

                def emit_scores(hp, j, i):
                    """Scores + exp for k-tile i of q-block j.  Diag tiles
                    are causal-masked by prefilling the psum with -1e30
                    above the diagonal (identity matmul on trib)."""
                    r = i - 4 * j
                    offs = 128 * r if r >= 0 else 0
                    sc = psA.tile([128, 1024], F32, tag="big", bufs=3,
                                  name="sc")
                    pt = ptpool.tile([128, 1024], BF, tag="pt", bufs=5,
                                     name="pt")
                    for hh in range(2):
                        nc.tensor.matmul(
                            sc[:, hh * 512 + offs:hh * 512 + 512],
                            krot[hp][hh * D:hh * D + D,
                                     i * 128:(i + 1) * 128],
                            qrot[hp][hh * D:hh * D + D,
                                     j * 512 + offs:(j + 1) * 512],
                            start=True, stop=(r < 0),
                            tile_position=(hh * D, 0))
                    if r >= 0:
                        for hh in range(2):
                            nc.tensor.matmul(
                                sc[:, hh * 512 + offs:hh * 512 + offs + 128],
                                iden_t[:, :], trib_t[:, 0:128],
                                start=False, stop=True)
                    src = sc[:, :].rearrange("p (h x) -> p h x",
                                             h=2)[:, :, offs:512]
                    dstv = pt[:, :].rearrange("p (h x) -> p h x",
                                              h=2)[:, :, offs:512]
                    nc.scalar.activation(
                        dstv, src, mybir.ActivationFunctionType.Exp,
                        scale=float(scale))
                    return pt, offs

                def emit_pv(hp, j, i, pt, offs, n_i):
                    if i == 0:
                        po_cur[0] = psA.tile([128, 1024], F32, tag="po",
                                             bufs=1, name="po")
                    po = po_cur[0]
                    for hh in range(2):
                        h = 2 * hp + hh
                        nc.tensor.matmul(
                            po[0:65, hh * 512 + offs:hh * 512 + 512],
                            v_t[i][:, h * 65:h * 65 + 65],
                            pt[:, hh * 512 + offs:hh * 512 + 512],
                            start=(i == 0), stop=(i == n_i - 1))
                    if i == n_i - 1:
                        finish_block(hp, j, po)

                po_cur = [None]

                def queue_outproj(j):
                    """Push outproj(j) as 4 filler thunks (one per e-chunk)."""
                    def mk(j, eb):
                        def f():
                            pp = psA.tile([128, 1024], F32, tag="big",
                                          bufs=3, name="pp")
                            for f2 in range(2):
                                nc.tensor.matmul(
                                    pp[:, 0:512],
                                    wo_t[f2][:, eb * 128:(eb + 1) * 128],
                                    onorm[(j, f2)][:, :],
                                    start=(f2 == 0), stop=(f2 == 1))
                            oc = ocpool.tile([128, 512], BF,
                                             tag="oc", name="oc")
                            nc.vector.tensor_copy(oc[:, :], pp[:, 0:512])
                            nc.sync.dma_start(
                                outT[eb * 128:(eb + 1) * 128,
                                     j * 512:(j + 1) * 512],
                                oc[:, :])
                        return f
                    for eb in range(4):
                        filler.append(mk(j, eb))

                def queue_norm_tail(j, hp, praw, rrow_b):
                    """Deferred: broadcast 1/denom via ones-matmul into a
                    fresh duo-psum tile, multiply praw by it into onorm."""
                    def f():
                        onj = onpool.tile([128, 512], BF, tag=f"on{j}{hp}",
                                          name=f"on{j}{hp}")
                        onorm[(j, hp)] = onj
                        bc = psA.tile([128, 1024], F32, tag="big", bufs=3,
                                      name="bc")
                        for hh in range(2):
                            nc.tensor.matmul(
                                bc[0:D, hh * 512:hh * 512 + 512],
                                ones_b[:, :],
                                rrow_b[:, hh * 512:hh * 512 + 512],
                                start=True, stop=True)
                        for hh in range(2):
                            nc.vector.tensor_mul(
                                onj[hh * D:hh * D + D, :],
                                praw[0:D, hh * 512:hh * 512 + 512],
                                bc[0:D, hh * 512:hh * 512 + 512])
                        if debug and j == 1 and hp == 0:
                            nc.sync.dma_start(dbg_on[:, :], onj[:, :])
                        if hp == 1:
                            queue_outproj(j)
                    filler.append(f)

                def finish_block(hp, j, po):
                    # free po fast: copy raw PV (rows 0:64 per head + denom
                    # row 64) to SBUF, compute 1/denom = Exp(-Ln(denom)) on
                    # the scalar engine; the broadcast + normalize multiply
                    # are deferred into the filler queue
                    praw = spool.tile([65, 1024], BF, tag=f"praw{j}{hp}",
                                      name="praw")
                    nc.vector.tensor_copy(praw[:, :], po[0:65, :])
                    rln = spool.tile([1, 1024], F32, tag="rln", name="rln")
                    rrow_b = spool.tile([1, 1024], BF, tag=f"rrowb{j}{hp}",
                                        name="rrowb")
                    nc.scalar.activation(rln[:, :], praw[64:65, :],
                                         mybir.ActivationFunctionType.Ln)
                    nc.scalar.activation(rrow_b[:, :], rln[:, :],
                                         mybir.ActivationFunctionType.Exp,
                                         scale=-1.0)
                    queue_norm_tail(j, hp, praw, rrow_b)

                # schedule: stack-1 projections, normalize tails and outproj
                # run as fillers inside one GLOBAL software-pipelined step
                # stream (scores lookahead 2, PV behind) spanning all eight
                # (stack, q-block) attention blocks, so the PE never drains
                project_rope("k", HG * D, krot, 0)
                project_rope("q", 0, qrot, 0)
                project_v()
                project_rope("k", HG * D, krot, 1, queued=True)
                project_rope("q", 0, qrot, 1, queued=True)
                SCHED = ((0, 1), (0, 2), (0, 3), (1, 1), (0, 0),
                         (1, 2), (1, 0), (1, 3))
                steps = [(hp, j, i) for hp, j in SCHED
                         for i in range(4 * j + 4)]
                pts = {}
                for idx in range(len(steps)):
                    if idx == 0:
                        pts[steps[0]] = emit_scores(*steps[0])
                        pts[steps[1]] = emit_scores(*steps[1])
                    if idx + 2 < len(steps):
                        pts[steps[idx + 2]] = emit_scores(*steps[idx + 2])
                    drain_filler()
                    hp, j, i = steps[idx]
                    pt, offs = pts.pop((hp, j, i))
                    emit_pv(hp, j, i, pt, offs, 4 * j + 4)
                while filler:
                    filler.pop(0)()
                if debug:
                    nc.sync.dma_start(dbg_qrot[:, :], qrot[0][:, :])
                    nc.sync.dma_start(dbg_krot[:, :], krot[0][:, :])
                    nc.sync.dma_start(dbg_vt[:, :], v_t[0][:, :])
    return _split_excess_waits(nc)


_NC_CACHE = {}


def _get_nc(reps=1):
    if reps not in _NC_CACHE:
        _NC_CACHE[reps] = build_nc(reps)
    return _NC_CACHE[reps]


_RUNNER_CACHE = {}


def _get_runner(nc, n_cores):
    """Clone of bass2jax.run_bass_via_pjrt's multi-core path with the
    jitted callable cached so repeat calls skip retracing."""
    key = id(nc)
    if key in _RUNNER_CACHE:
        return _RUNNER_CACHE[key]
    import jax
    from jax.sharding import Mesh, PartitionSpec
    from jax.experimental.shard_map import shard_map
    from concourse import bass2jax as b2j

    b2j.install_neuronx_cc_hook()
    partition_name = (nc.partition_id_tensor.name
                      if nc.partition_id_tensor else None)
    in_names, out_names, out_avals, zero_outs = [], [], [], []
    for alloc in nc.m.functions[0].allocations:
        if not isinstance(alloc, mybir.MemoryLocationSet):
            continue
        name = alloc.memorylocations[0].name
        if alloc.kind == "ExternalInput":
            if name != partition_name:
                in_names.append(name)
        elif alloc.kind == "ExternalOutput":
            shape = tuple(alloc.tensor_shape)
            dtype = mybir.dt.np(alloc.dtype)
            out_names.append(name)
            out_avals.append(jax.core.ShapedArray(shape, dtype))
            zero_outs.append(np.zeros(shape, dtype))
    n_params = len(in_names)
    n_outs = len(out_avals)
    in_names_all = list(in_names) + list(out_names)
    if partition_name is not None:
        in_names_all.append(partition_name)
    donate = tuple(range(n_params, n_params + n_outs))

    def _body(*args):
        operands = list(args)
        if partition_name is not None:
            operands.append(b2j.partition_id_tensor())
        outs = b2j._bass_exec_p.bind(
            *operands,
            out_avals=tuple(out_avals),
            in_names=tuple(in_names_all),
            out_names=tuple(out_names),
            lowering_input_output_aliases=(),
            sim_require_finite=True,
            sim_require_nnan=True,
            nc=nc,
        )
        return tuple(outs)

    devices = jax.devices()[:n_cores]
    mesh = Mesh(np.asarray(devices), ("core",))
    in_specs = (PartitionSpec("core"),) * (n_params + n_outs)
    out_specs = (PartitionSpec("core"),) * len(out_names)
    sharded = jax.jit(
        shard_map(_body, mesh=mesh, in_specs=in_specs, out_specs=out_specs,
                  check_rep=False),
        donate_argnums=donate, keep_unused=True)

    def run(in_maps):
        gins = [np.concatenate([np.asarray(m[name]) for m in in_maps], axis=0)
                for name in in_names]
        gzeros = [np.concatenate([z] * n_cores, axis=0) for z in zero_outs]
        outs = sharded(*gins, *gzeros)
        res = []
        for c in range(n_cores):
            res.append({})
        for i, name in enumerate(out_names):
            arr = np.asarray(outs[i])
            per = arr.shape[0] // n_cores
            for c in range(n_cores):
                res[c][name] = arr[c * per:(c + 1) * per]
        return res

    _RUNNER_CACHE[key] = run
    return run


def _make_in_maps(inputs, wqkv_w, wqkv_b, wo_w):
    x = np.asarray(inputs, np.float32).reshape(B * V, S, E)
    wcache = {}
    in_maps = []
    xTb = {}
    for c in range(NCORE):
        g, hg = c // 2, c % 2
        if hg not in wcache:
            wcache[hg] = _host_weights(wqkv_w, wqkv_b, wo_w, hg)
        if g not in xTb:
            xTb[g] = _bf16(np.ascontiguousarray(x[g].T))
        wd = wcache[hg]
        in_maps.append(dict(
            xT=xTb[g], wqkvT=wd["wqkvT"], woT=wd["woT"],
            cs=wd["cs"], snS=wd["snS"], trib=wd["trib"],
            iden=wd["iden"]))
    return in_maps


def kernel(layer_idx=None, inputs=None, wqkv_w=None, wqkv_b=None,
           wo_w=None, wo_b=None):
    wqkv_w = np.asarray(wqkv_w, dtype=np.float32)
    wqkv_b = np.asarray(wqkv_b, dtype=np.float32)
    wo_w = np.asarray(wo_w, dtype=np.float32)
    wo_b = np.asarray(wo_b, dtype=np.float32)
    assert not np.any(wqkv_b), "nonzero wqkv_b not supported by this kernel build"

    nc = _get_nc()
    in_maps = _make_in_maps(inputs, wqkv_w, wqkv_b, wo_w)
    run = _get_runner(nc, NCORE)
    outs = run(in_maps)
    y = np.empty((B * V, S, E), dtype=np.float32)
    for g in range(B * V):
        acc = (outs[2 * g]["outT"].astype(np.float32)
               + outs[2 * g + 1]["outT"].astype(np.float32))   # (E, S)
        y[g] = acc.T
    y += wo_b[None, None, :]
    return y.reshape(B, V, S, E)
